# revision 14
# baseline (speedup 1.0000x reference)
"""Trainium2 Bass kernel: 16-head MHA (B=2, T=2048, D=1024), head-TP over 8 cores.

Per core c: heads 2c, 2c+1 (128 channels). Device computes x@Wqkv(+b) for its
head slice, scoresT=K@Q^T (scale folded into Wq), exp via ACT, P@V with an
appended ones-column producing the softmax denominator for free, normalize,
then partial proj = attn_c @ Wproj[c-slice]. Host sums the 8 partials + b_proj.
"""

import numpy as np
import ml_dtypes
from contextlib import ExitStack

B, T, C = 2, 2048, 1024
H, DH = 16, 64
NCORES = 8
CH = 128               # channels per core = 2 heads
NTOK = B * T           # 4096
NKC = T // 128         # 16 key chunks per batch
NQC = T // 512         # 4 query chunks per batch
SCALE = DH ** -0.5

_CACHE = {}


def _build(debug=False):
    import concourse.bass as bass  # noqa: F401
    import concourse.bacc as bacc
    import concourse.mybir as mybir
    import concourse.tile as tile

    f32 = mybir.dt.float32
    bf16 = mybir.dt.bfloat16
    EXP = mybir.ActivationFunctionType.Exp

    # Bacc (not Bass): its compile() runs move_matmul_waits_to_ldweights +
    # generate_event_semaphores, without which walrus rejects matmuls
    # carrying 2 sync waits ("Too many sync wait commands").
    nc = bacc.Bacc("TRN2", target_bir_lowering=False, debug=False)
    xT_d = nc.declare_dram_parameter("xT", [C, NTOK], bf16, isOutput=False)
    wq_d = nc.declare_dram_parameter("wq", [128, C], bf16, isOutput=False)
    wk_d = nc.declare_dram_parameter("wk", [128, C], bf16, isOutput=False)
    wv_d = nc.declare_dram_parameter("wv", [128, C], bf16, isOutput=False)
    wp_d = nc.declare_dram_parameter("wp", [CH, C], bf16, isOutput=False)
    bq_d = nc.declare_dram_parameter("bq", [1, CH], bf16, isOutput=False)
    bk_d = nc.declare_dram_parameter("bk", [1, CH], bf16, isOutput=False)
    bv_d = nc.declare_dram_parameter("bv", [1, CH], bf16, isOutput=False)
    out_d = nc.declare_dram_parameter("out", [NTOK, C], f32, isOutput=True)
    if debug:
        dbg = {
            "qT": nc.declare_dram_parameter("qT_dbg", [128, NTOK], bf16, isOutput=True),
            "kT": nc.declare_dram_parameter("kT_dbg", [128, NTOK], bf16, isOutput=True),
            "v0": nc.declare_dram_parameter("v0_dbg", [128, B * NKC * 65], bf16, isOutput=True),
            "exp": nc.declare_dram_parameter("exp_dbg", [128, 1024], bf16, isOutput=True),
            "d": nc.declare_dram_parameter("d_dbg", [1, 1024], f32, isOutput=True),
            "r": nc.declare_dram_parameter("r_dbg", [1, 1024], f32, isOutput=True),
            "bc": nc.declare_dram_parameter("bc_dbg", [64, 1024], f32, isOutput=True),
            "at": nc.declare_dram_parameter("at_dbg", [CH, T], bf16, isOutput=True),
        }

    with tile.TileContext(nc) as tc, ExitStack() as ctx:
        ep = ctx.enter_context

        # ---------------- persistent SBUF ----------------
        xT_pool = ep(tc.tile_pool(name="xT", bufs=8))
        xT_sb = [xT_pool.tile([128, NTOK], bf16, name=f"xT{k}", tag="xT") for k in range(8)]
        w_pool = ep(tc.tile_pool(name="w", bufs=4))
        wq_sb = w_pool.tile([128, C], bf16, tag="wq")
        wk_sb = w_pool.tile([128, C], bf16, tag="wk")
        wv_sb = w_pool.tile([128, C], bf16, tag="wv")
        wp_sb = w_pool.tile([CH, C], bf16, tag="wp")
        b_pool = ep(tc.tile_pool(name="bias", bufs=3))
        bq_sb = b_pool.tile([1, CH], bf16, tag="bq")
        bk_sb = b_pool.tile([1, CH], bf16, tag="bk")
        bv_sb = b_pool.tile([1, CH], bf16, tag="bv")
        const_pool = ep(tc.tile_pool(name="const", bufs=2))
        ones_bf = const_pool.tile([1, 512], bf16, tag="ones_bf")
        onesf = const_pool.tile([65, 64], f32, tag="onesf")  # row 64 used
        qk_pool = ep(tc.tile_pool(name="qk", bufs=2))
        qT_sb = qk_pool.tile([CH, NTOK], bf16, tag="qT")
        kT_sb = qk_pool.tile([CH, NTOK], bf16, tag="kT")
        v_pool = ep(tc.tile_pool(name="v", bufs=2))
        # per head: B*NKC chunks of [128 keys, 64 feats + ones col]
        v_sb = [v_pool.tile([128, B * NKC * 65], bf16, name=f"v{h}", tag="v") for h in range(2)]
        attn_pool = ep(tc.tile_pool(name="attn", bufs=2))
        attnT = [attn_pool.tile([CH, T], bf16, name=f"attnT{b}", tag="attnT") for b in range(B)]
        attn1_pool = ep(tc.tile_pool(name="attn1", bufs=2))
        attnT_h1 = [attn1_pool.tile([64, T], bf16, name=f"attnTh1{b}", tag="attnTh1") for b in range(B)]
        exp_pool = ep(tc.tile_pool(name="exp", bufs=3))
        d_pool = ep(tc.tile_pool(name="dsb", bufs=2))
        bc_pool = ep(tc.tile_pool(name="bcsb", bufs=1))
        out_pool = ep(tc.tile_pool(name="outsb", bufs=4))

        # ---------------- load inputs ----------------
        for k in range(8):
            nc.sync.dma_start(xT_sb[k][:], xT_d[k * 128:(k + 1) * 128, :])
        nc.gpsimd.dma_start(wq_sb[:], wq_d[:])
        nc.gpsimd.dma_start(wk_sb[:], wk_d[:])
        nc.gpsimd.dma_start(wv_sb[:], wv_d[:])
        nc.gpsimd.dma_start(wp_sb[:], wp_d[:])
        nc.gpsimd.dma_start(bq_sb[:], bq_d[:])
        nc.gpsimd.dma_start(bk_sb[:], bk_d[:])
        nc.gpsimd.dma_start(bv_sb[:], bv_d[:])
        nc.vector.memset(ones_bf[:], 1.0)
        nc.vector.memset(onesf[:], 1.0)
        # ones column at col 64 of every 65-wide v chunk
        for h in range(2):
            nc.vector.memset(v_sb[h][:, 64::65], 1.0)

        # ---------------- phase A: qkv projections ----------------
        with tc.tile_pool(name="qkv_ps", bufs=4, space="PSUM") as qkv_ps:
            # qT/kT: [CH feats, NTOK tokens], 512-token chunks, 8+1 matmuls each
            for w_sb, bias_sb, dst in ((wq_sb, bq_sb, qT_sb), (wk_sb, bk_sb, kT_sb)):
                for t in range(NTOK // 512):
                    ps = qkv_ps.tile([CH, 512], f32, name="qk_ps", tag="qk_ps")
                    for k in range(8):
                        nc.tensor.matmul(
                            ps[:], lhsT=w_sb[:, k * 128:(k + 1) * 128],
                            rhs=xT_sb[k][:, t * 512:(t + 1) * 512],
                            start=(k == 0), stop=False)
                    nc.tensor.matmul(
                        ps[:], lhsT=bias_sb[:], rhs=ones_bf[:],
                        start=False, stop=True)
                    nc.vector.tensor_copy(dst[:, t * 512:(t + 1) * 512], ps[:])
            # v: [token, feat] 128-token chunks; split per head into v_sb
            for t in range(NTOK // 128):
                ps = qkv_ps.tile([128, CH], f32, name="v_ps", tag="v_ps")
                for k in range(8):
                    nc.tensor.matmul(
                        ps[:], lhsT=xT_sb[k][:, t * 128:(t + 1) * 128],
                        rhs=wv_sb[:, k * 128:(k + 1) * 128],
                        start=(k == 0), stop=False)
                nc.tensor.matmul(
                    ps[:], lhsT=ones_bf[:, :128], rhs=bv_sb[:],
                    start=False, stop=True)
                for h in range(2):
                    nc.vector.tensor_copy(
                        v_sb[h][:, t * 65:t * 65 + 64], ps[:, h * 64:(h + 1) * 64])

        # ---------------- phase B: attention (+ interleaved proj of prev b) ---
        def emit_proj(b, tcs):
            """proj partial for token-chunks tcs of batch b: out += attn @ Wp_c"""
            for tci in tcs:
                pps = scores_ps.tile([128, 1024], f32, name="proj_ps", tag="ps")
                osb = out_pool.tile([128, 1024], f32, name="out_sb", tag="out_sb")
                for ncol in range(2):
                    nc.tensor.matmul(
                        pps[:, ncol * 512:(ncol + 1) * 512],
                        lhsT=attnT[b][:, tci * 128:(tci + 1) * 128],
                        rhs=wp_sb[:, ncol * 512:(ncol + 1) * 512],
                        start=True, stop=True)
                nc.vector.tensor_copy(osb[:], pps[:])
                nc.gpsimd.dma_start(
                    out_d[b * T + tci * 128: b * T + (tci + 1) * 128, :], osb[:])

        with tc.tile_pool(name="scores_ps", bufs=3, space="PSUM") as scores_ps, \
             tc.tile_pool(name="pv_ps", bufs=2, space="PSUM") as pv_ps:
            for b in range(B):
                for qc in range(NQC):
                    q_sl = slice(b * T + qc * 512, b * T + (qc + 1) * 512)
                    pv = [pv_ps.tile([65, 512], f32, name=f"pv{h}", tag="pv") for h in range(2)]
                    sc_tiles = {}
                    exp_tiles = {}

                    def emit_scores(kc):
                        sc = scores_ps.tile([128, 1024], f32, name="sc_ps", tag="ps")
                        k_sl = slice(b * T + kc * 128, b * T + (kc + 1) * 128)
                        for h in range(2):
                            nc.tensor.matmul(
                                sc[:, h * 512:(h + 1) * 512],
                                lhsT=kT_sb[h * 64:(h + 1) * 64, k_sl],
                                rhs=qT_sb[h * 64:(h + 1) * 64, q_sl],
                                start=True, stop=True)
                        ex = exp_pool.tile([128, 1024], bf16, name="exp_sb", tag="exp_sb")
                        nc.scalar.activation(ex[:], sc[:], EXP)
                        sc_tiles[kc] = sc
                        exp_tiles[kc] = ex
                        if debug and b == 0 and qc == 0 and kc == 0:
                            nc.gpsimd.dma_start(dbg["exp"][:], ex[:])

                    def emit_pv(kc):
                        gkc = b * NKC + kc
                        ex = exp_tiles.pop(kc)
                        for h in range(2):
                            nc.tensor.matmul(
                                pv[h][:],
                                lhsT=v_sb[h][:, gkc * 65:gkc * 65 + 65],
                                rhs=ex[:, h * 512:(h + 1) * 512],
                                start=(kc == 0), stop=(kc == NKC - 1),
                                skip_group_check=True)

                    # software-pipelined: scores run 2 ahead of PV
                    for kc in range(NKC):
                        emit_scores(kc)
                        if kc >= 2:
                            emit_pv(kc - 2)
                    emit_pv(NKC - 2)
                    emit_pv(NKC - 1)

                    # normalize: broadcast denominator row, recip, multiply
                    dsb = d_pool.tile([65, 1024], f32, name="d_sb", tag="d_sb")
                    for h in range(2):
                        nc.vector.tensor_copy(
                            dsb[64:65, h * 512:(h + 1) * 512], pv[h][64:65, :])
                    bc_ps = scores_ps.tile([128, 1024], f32, name="bc_ps", tag="ps")
                    bcsb = bc_pool.tile([64, 1024], f32, name="bc_sb", tag="bc_sb")
                    for h in range(2):
                        nc.tensor.matmul(
                            bc_ps[0:64, h * 512:(h + 1) * 512],
                            lhsT=onesf[64:65, :],
                            rhs=dsb[64:65, h * 512:(h + 1) * 512],
                            start=True, stop=True)
                    nc.vector.reciprocal_approx_fast(
                        out=bcsb[:], in_=bc_ps[0:64, :])
                    qcs = slice(qc * 512, (qc + 1) * 512)
                    nc.vector.tensor_mul(
                        attnT[b][0:64, qcs], pv[0][0:64, :], bcsb[:, 0:512])
                    nc.vector.tensor_mul(
                        attnT_h1[b][:, qcs], pv[1][0:64, :], bcsb[:, 512:1024])
                    if debug and b == 0 and qc == 0:
                        nc.gpsimd.dma_start(dbg["d"][:], dsb[64:65, :])
                        nc.gpsimd.dma_start(dbg["r"][:], bcsb[0:1, :])
                        nc.gpsimd.dma_start(dbg["bc"][:], bcsb[:])

                    # interleave previous batch's proj into this batch's PE gaps
                    if b == 1:
                        emit_proj(0, range(qc * 4, (qc + 1) * 4))

                # h1 rows into partitions 64-127 of attnT via SBUF->SBUF DMA
                nc.sync.dma_start(attnT[b][64:CH, :], attnT_h1[b][:])
                if b == 1:
                    emit_proj(1, range(16))
                if debug and b == 0:
                    nc.gpsimd.dma_start(dbg["at"][:], attnT[0][:])

        if debug:
            nc.gpsimd.dma_start(dbg["qT"][:], qT_sb[:])
            nc.gpsimd.dma_start(dbg["kT"][:], kT_sb[:])
            nc.gpsimd.dma_start(dbg["v0"][:], v_sb[0][:])

    nc.compile()
    return nc


def _prep_inputs(x, W_qkv, b_qkv, W_proj, b_proj):
    bf = ml_dtypes.bfloat16
    xT = np.ascontiguousarray(
        x.reshape(NTOK, C).T).astype(bf)
    in_maps = []
    for c in range(NCORES):
        cs = slice(c * CH, (c + 1) * CH)
        wq = np.ascontiguousarray(
            (W_qkv[:, c * CH:(c + 1) * CH] * SCALE)
            .reshape(8, 128, CH).transpose(1, 0, 2).reshape(128, C)).astype(bf)
        wk = np.ascontiguousarray(
            W_qkv[:, C + c * CH:C + (c + 1) * CH]
            .reshape(8, 128, CH).transpose(1, 0, 2).reshape(128, C)).astype(bf)
        wv = np.ascontiguousarray(
            W_qkv[:, 2 * C + c * CH:2 * C + (c + 1) * CH]
            .reshape(8, 128, CH).transpose(1, 0, 2).reshape(128, C)).astype(bf)
        wp = np.ascontiguousarray(W_proj[cs, :]).astype(bf)
        bq = (b_qkv[c * CH:(c + 1) * CH] * SCALE).reshape(1, CH).astype(bf)
        bk = b_qkv[C + c * CH:C + (c + 1) * CH].reshape(1, CH).astype(bf)
        bv = b_qkv[2 * C + c * CH:2 * C + (c + 1) * CH].reshape(1, CH).astype(bf)
        in_maps.append({
            "xT": xT, "wq": wq, "wk": wk, "wv": wv, "wp": wp,
            "bq": bq, "bk": bk, "bv": bv,
        })
    return in_maps


def _run(inputs, trace=False):
    from concourse import bass_utils
    if "nc" not in _CACHE:
        _CACHE["nc"] = _build()
    nc = _CACHE["nc"]
    in_maps = _prep_inputs(
        np.asarray(inputs["x"], np.float32),
        np.asarray(inputs["W_qkv"], np.float32),
        np.asarray(inputs["b_qkv"], np.float32),
        np.asarray(inputs["W_proj"], np.float32),
        np.asarray(inputs["b_proj"], np.float32),
    )
    br = bass_utils.run_bass_kernel_spmd(
        nc, in_maps, core_ids=list(range(NCORES)), trace=trace)
    partial = np.zeros((NTOK, C), np.float64)
    for r in br.results:
        partial += np.asarray(r["out"], np.float32).astype(np.float64)
    out = (partial + np.asarray(inputs["b_proj"], np.float64)[None, :]).astype(
        np.float32).reshape(B, T, C)
    return out, br


def kernel(**inputs) -> np.ndarray:
    out, _ = _run(inputs, trace=False)
    return out


# revision 36
# speedup vs baseline: 1.2980x; 1.2980x over previous
"""Trainium2 Bass kernel: 16-head MHA (B=2, T=2048, D=1024), head-TP over 8 cores.

Per core c: heads 2c, 2c+1 (128 channels). Device computes x@Wqkv(+b) for its
head slice, scoresT=K@Q^T (scale folded into Wq), exp via ACT, P@V with an
appended ones-column producing the softmax denominator for free, normalize,
then partial proj = attn_c @ Wproj[c-slice]. Host sums the 8 partials + b_proj.
"""

import numpy as np
import ml_dtypes
from contextlib import ExitStack

B, T, C = 2, 2048, 1024
H, DH = 16, 64
NCORES = 8
CH = 128               # channels per core = 2 heads
NTOK = B * T           # 4096
NKC = T // 128         # 16 key chunks per batch
NQC = T // 512         # 4 query chunks per batch
SCALE = DH ** -0.5

_CACHE = {}


def _build(debug=False):
    import concourse.bass as bass  # noqa: F401
    import concourse.bacc as bacc
    import concourse.mybir as mybir
    import concourse.tile as tile

    f32 = mybir.dt.float32
    bf16 = mybir.dt.bfloat16
    EXP = mybir.ActivationFunctionType.Exp
    IDENT = mybir.ActivationFunctionType.Identity

    # Bacc (not Bass): its compile() runs move_matmul_waits_to_ldweights +
    # generate_event_semaphores, without which walrus rejects matmuls
    # carrying 2 sync waits ("Too many sync wait commands").
    nc = bacc.Bacc("TRN2", target_bir_lowering=False, debug=False)
    xT_d = nc.declare_dram_parameter("xT", [C, NTOK], bf16, isOutput=False)
    wq_d = nc.declare_dram_parameter("wq", [128, C], bf16, isOutput=False)
    wk_d = nc.declare_dram_parameter("wk", [128, C], bf16, isOutput=False)
    wv_d = nc.declare_dram_parameter("wv", [128, C], bf16, isOutput=False)
    wp_d = nc.declare_dram_parameter("wp", [CH, C], bf16, isOutput=False)
    bqc_d = nc.declare_dram_parameter("bqc", [CH, 1], f32, isOutput=False)
    bkc_d = nc.declare_dram_parameter("bkc", [CH, 1], f32, isOutput=False)
    bv_d = nc.declare_dram_parameter("bv", [1, CH], bf16, isOutput=False)
    out_d = nc.declare_dram_parameter("out", [NTOK, C], bf16, isOutput=True)
    if debug:
        dbg = {
            "qT": nc.declare_dram_parameter("qT_dbg", [128, NTOK], bf16, isOutput=True),
            "kT": nc.declare_dram_parameter("kT_dbg", [128, NTOK], bf16, isOutput=True),
            "v0": nc.declare_dram_parameter("v0_dbg", [128, B * NKC * 128], bf16, isOutput=True),
            "exp": nc.declare_dram_parameter("exp_dbg", [128, 1024], bf16, isOutput=True),
            "d": nc.declare_dram_parameter("d_dbg", [1, 1024], f32, isOutput=True),
            "r": nc.declare_dram_parameter("r_dbg", [1, 1024], f32, isOutput=True),
            "bc": nc.declare_dram_parameter("bc_dbg", [64, 1024], f32, isOutput=True),
            "at": nc.declare_dram_parameter("at_dbg", [CH, T], bf16, isOutput=True),
        }

    with tile.TileContext(nc) as tc, ExitStack() as ctx:
        ep = ctx.enter_context

        # ---------------- persistent SBUF ----------------
        xT_pool = ep(tc.tile_pool(name="xT", bufs=8))
        xT_sb = [xT_pool.tile([128, NTOK], bf16, name=f"xT{k}", tag="xT") for k in range(8)]
        w_pool = ep(tc.tile_pool(name="w", bufs=4))
        wq_sb = w_pool.tile([128, C], bf16, tag="wq")
        wk_sb = w_pool.tile([128, C], bf16, tag="wk")
        wv_sb = w_pool.tile([128, C], bf16, tag="wv")
        wp_sb = w_pool.tile([CH, C], bf16, tag="wp")
        b_pool = ep(tc.tile_pool(name="bias", bufs=1))
        bqc_sb = b_pool.tile([CH, 1], f32, tag="bqc")
        bkc_sb = b_pool.tile([CH, 1], f32, tag="bkc")
        bv_sb = b_pool.tile([1, CH], bf16, tag="bv")
        bv_bc = b_pool.tile([128, CH], bf16, tag="bv_bc")
        const_pool = ep(tc.tile_pool(name="const", bufs=2))
        ones_bf = const_pool.tile([1, 512], bf16, tag="ones_bf")
        qk_pool = ep(tc.tile_pool(name="qk", bufs=2))
        qT_sb = qk_pool.tile([CH, NTOK], bf16, tag="qT")
        kT_sb = qk_pool.tile([CH, NTOK], bf16, tag="kT")
        v_pool = ep(tc.tile_pool(name="v", bufs=2))
        # per head: B*NKC chunks of [128 keys, 64 ones cols | 64 feats]; the
        # ones cols make the PV matmul replicate the softmax denominator onto
        # output partitions 0:64 for free (recip reads physical partition 0).
        v_sb = [v_pool.tile([128, B * NKC * 128], bf16, name=f"v{h}", tag="v") for h in range(2)]
        attn_pool = ep(tc.tile_pool(name="attn", bufs=2))
        attnT = [attn_pool.tile([CH, T], bf16, name=f"attnT{b}", tag="attnT") for b in range(B)]
        attn1_pool = ep(tc.tile_pool(name="attn1", bufs=2))
        attnT_h1 = [attn1_pool.tile([64, T], bf16, name=f"attnTh1{b}", tag="attnTh1") for b in range(B)]
        exp_pool = ep(tc.tile_pool(name="exp", bufs=3))
        bc_pool = ep(tc.tile_pool(name="bcsb", bufs=1))
        out_pool = ep(tc.tile_pool(name="outsb", bufs=4))

        # ---------------- load inputs ----------------
        nc.gpsimd.dma_start(wq_sb[:], wq_d[:])
        nc.gpsimd.dma_start(wk_sb[:], wk_d[:])
        nc.gpsimd.dma_start(wv_sb[:], wv_d[:])
        nc.gpsimd.dma_start(wp_sb[:], wp_d[:])
        nc.gpsimd.dma_start(bqc_sb[:], bqc_d[:])
        nc.gpsimd.dma_start(bkc_sb[:], bkc_d[:])
        nc.gpsimd.dma_start(bv_sb[:], bv_d[:])
        # x column-chunk DMAs in consumption order so phase A starts early
        for t in range(8):
            for k in range(8):
                nc.sync.dma_start(
                    xT_sb[k][:, t * 512:(t + 1) * 512],
                    xT_d[k * 128:(k + 1) * 128, t * 512:(t + 1) * 512])
        nc.vector.memset(ones_bf[:], 1.0)
        # whole v tile to 1.0; value cols 0:64 of each chunk overwritten later
        for h in range(2):
            nc.vector.memset(v_sb[h][:], 1.0)

        # ---------------- phase A: qkv projections ----------------
        with tc.tile_pool(name="qkv_ps", bufs=4, space="PSUM") as qkv_ps:
            # bv broadcast [128, CH] built once (v bias folded into DVE copy)
            bvps = qkv_ps.tile([128, CH], f32, name="bv_ps", tag="v_ps")
            nc.tensor.matmul(bvps[:], lhsT=ones_bf[:, :128], rhs=bv_sb[:],
                             start=True, stop=True)
            nc.vector.tensor_copy(bv_bc[:], bvps[:])
            # per 512-token group: q chunk, k chunk, then 4 v chunks
            for t in range(NTOK // 512):
                for w_sb, bias_col, dst in ((wq_sb, bqc_sb, qT_sb),
                                            (wk_sb, bkc_sb, kT_sb)):
                    ps = qkv_ps.tile([CH, 512], f32, name="qk_ps", tag="qk_ps")
                    for k in range(8):
                        nc.tensor.matmul(
                            ps[:], lhsT=w_sb[:, k * 128:(k + 1) * 128],
                            rhs=xT_sb[k][:, t * 512:(t + 1) * 512],
                            start=(k == 0), stop=(k == 7))
                    # bias add fused into PSUM->SBUF copy on the (idle) ACT engine
                    nc.scalar.activation(
                        dst[:, t * 512:(t + 1) * 512], ps[:], IDENT,
                        bias=bias_col[:])
                for tt in range(t * 4, (t + 1) * 4):
                    ps = qkv_ps.tile([128, CH], f32, name="v_ps", tag="v_ps")
                    for k in range(8):
                        nc.tensor.matmul(
                            ps[:], lhsT=xT_sb[k][:, tt * 128:(tt + 1) * 128],
                            rhs=wv_sb[:, k * 128:(k + 1) * 128],
                            start=(k == 0), stop=(k == 7))
                    for h in range(2):
                        nc.vector.tensor_add(
                            v_sb[h][:, tt * 128 + 64:(tt + 1) * 128],
                            ps[:, h * 64:(h + 1) * 64],
                            bv_bc[:, h * 64:(h + 1) * 64])

        # ---------------- phase B: attention (+ interleaved proj of prev b) ---
        def emit_proj(b, tcs):
            """proj partial for token-chunks tcs of batch b: out += attn @ Wp_c"""
            for tci in tcs:
                pps = scores_ps.tile([128, 1024], f32, name="proj_ps", tag="ps")
                osb = out_pool.tile([128, 1024], bf16, name="out_sb", tag="out_sb")
                for ncol in range(2):
                    nc.tensor.matmul(
                        pps[:, ncol * 512:(ncol + 1) * 512],
                        lhsT=attnT[b][:, tci * 128:(tci + 1) * 128],
                        rhs=wp_sb[:, ncol * 512:(ncol + 1) * 512],
                        start=True, stop=True)
                nc.vector.tensor_copy(osb[:], pps[:])
                nc.gpsimd.dma_start(
                    out_d[b * T + tci * 128: b * T + (tci + 1) * 128, :], osb[:])

        with tc.tile_pool(name="scores_ps", bufs=3, space="PSUM") as scores_ps, \
             tc.tile_pool(name="pv_ps", bufs=2, space="PSUM") as pv_ps:
            for b in range(B):
                for qc in range(NQC):
                    q_sl = slice(b * T + qc * 512, b * T + (qc + 1) * 512)
                    pv = [pv_ps.tile([128, 512], f32, name=f"pv{h}", tag="pv") for h in range(2)]
                    sc_tiles = {}
                    exp_tiles = {}

                    def emit_scores(kc):
                        sc = scores_ps.tile([128, 1024], f32, name="sc_ps", tag="ps")
                        k_sl = slice(b * T + kc * 128, b * T + (kc + 1) * 128)
                        for h in range(2):
                            nc.tensor.matmul(
                                sc[:, h * 512:(h + 1) * 512],
                                lhsT=kT_sb[h * 64:(h + 1) * 64, k_sl],
                                rhs=qT_sb[h * 64:(h + 1) * 64, q_sl],
                                start=True, stop=True)
                        ex = exp_pool.tile([128, 1024], bf16, name="exp_sb", tag="exp_sb")
                        nc.scalar.activation(ex[:], sc[:], EXP)
                        sc_tiles[kc] = sc
                        exp_tiles[kc] = ex
                        if debug and b == 0 and qc == 0 and kc == 0:
                            nc.gpsimd.dma_start(dbg["exp"][:], ex[:])

                    def emit_pv(kc):
                        gkc = b * NKC + kc
                        ex = exp_tiles.pop(kc)
                        for h in range(2):
                            nc.tensor.matmul(
                                pv[h][:],
                                lhsT=v_sb[h][:, gkc * 128:(gkc + 1) * 128],
                                rhs=ex[:, h * 512:(h + 1) * 512],
                                start=(kc == 0), stop=(kc == NKC - 1),
                                skip_group_check=True)

                    # software-pipelined: scores run 2 ahead of PV
                    for kc in range(NKC):
                        emit_scores(kc)
                        if kc >= 2:
                            emit_pv(kc - 2)
                    emit_pv(NKC - 2)
                    emit_pv(NKC - 1)

                    # normalize: D replicated on pv partitions 0:64, PV on 64:128
                    bcsb = bc_pool.tile([64, 1024], f32, name="bc_sb", tag="bc_sb")
                    for h in range(2):
                        nc.vector.reciprocal_approx_fast(
                            out=bcsb[:, h * 512:(h + 1) * 512],
                            in_=pv[h][0:64, :])
                    qcs = slice(qc * 512, (qc + 1) * 512)
                    nc.vector.tensor_mul(
                        attnT[b][0:64, qcs], pv[0][64:128, :], bcsb[:, 0:512])
                    nc.vector.tensor_mul(
                        attnT_h1[b][:, qcs], pv[1][64:128, :], bcsb[:, 512:1024])
                    if debug and b == 0 and qc == 0:
                        nc.gpsimd.dma_start(dbg["r"][:], bcsb[0:1, :])
                        nc.gpsimd.dma_start(dbg["bc"][:], bcsb[:])

                    # h1 rows into partitions 64-127 of attnT via SBUF->SBUF DMA,
                    # then interleave proj of both batches into PE gaps
                    nc.sync.dma_start(attnT[b][64:CH, qcs], attnT_h1[b][:, qcs])
                    if b == 1:
                        emit_proj(0, range(qc * 4, (qc + 1) * 4))
                        emit_proj(1, range(qc * 4, (qc + 1) * 4))

                if debug and b == 0:
                    nc.gpsimd.dma_start(dbg["at"][:], attnT[0][:])

        if debug:
            nc.gpsimd.dma_start(dbg["qT"][:], qT_sb[:])
            nc.gpsimd.dma_start(dbg["kT"][:], kT_sb[:])
            nc.gpsimd.dma_start(dbg["v0"][:], v_sb[0][:])

    nc.compile()
    return nc


def _prep_inputs(x, W_qkv, b_qkv, W_proj, b_proj):
    bf = ml_dtypes.bfloat16
    xT = np.ascontiguousarray(
        x.reshape(NTOK, C).T).astype(bf)
    in_maps = []
    for c in range(NCORES):
        cs = slice(c * CH, (c + 1) * CH)
        wq = np.ascontiguousarray(
            (W_qkv[:, c * CH:(c + 1) * CH] * SCALE)
            .reshape(8, 128, CH).transpose(1, 0, 2).reshape(128, C)).astype(bf)
        wk = np.ascontiguousarray(
            W_qkv[:, C + c * CH:C + (c + 1) * CH]
            .reshape(8, 128, CH).transpose(1, 0, 2).reshape(128, C)).astype(bf)
        wv = np.ascontiguousarray(
            W_qkv[:, 2 * C + c * CH:2 * C + (c + 1) * CH]
            .reshape(8, 128, CH).transpose(1, 0, 2).reshape(128, C)).astype(bf)
        wp = np.ascontiguousarray(W_proj[cs, :]).astype(bf)
        bqc = (b_qkv[c * CH:(c + 1) * CH] * SCALE).reshape(CH, 1).astype(np.float32)
        bkc = b_qkv[C + c * CH:C + (c + 1) * CH].reshape(CH, 1).astype(np.float32)
        bv = b_qkv[2 * C + c * CH:2 * C + (c + 1) * CH].reshape(1, CH).astype(bf)
        in_maps.append({
            "xT": xT, "wq": wq, "wk": wk, "wv": wv, "wp": wp,
            "bqc": bqc, "bkc": bkc, "bv": bv,
        })
    return in_maps


def _run(inputs, trace=False):
    from concourse import bass_utils
    if "nc" not in _CACHE:
        _CACHE["nc"] = _build()
    nc = _CACHE["nc"]
    in_maps = _prep_inputs(
        np.asarray(inputs["x"], np.float32),
        np.asarray(inputs["W_qkv"], np.float32),
        np.asarray(inputs["b_qkv"], np.float32),
        np.asarray(inputs["W_proj"], np.float32),
        np.asarray(inputs["b_proj"], np.float32),
    )
    br = bass_utils.run_bass_kernel_spmd(
        nc, in_maps, core_ids=list(range(NCORES)), trace=trace)
    partial = np.zeros((NTOK, C), np.float64)
    for r in br.results:
        partial += np.asarray(r["out"]).astype(np.float64)
    out = (partial + np.asarray(inputs["b_proj"], np.float64)[None, :]).astype(
        np.float32).reshape(B, T, C)
    return out, br


def kernel(**inputs) -> np.ndarray:
    out, _ = _run(inputs, trace=False)
    return out
